# revision 40
# baseline (speedup 1.0000x reference)
"""Bass/Trainium2 kernel for nn_BaselineLSTM (B=2048, T=512, H=128, twin=256).

Strategy (final):
  - Data-parallel: batch 2048 -> 8 cores x 256; each core runs 2 interleaved
    chunks of 128 batch. The kernel is bound three ways at ~2.9us/step
    (per-chunk serial chain ~ PE busy ~ ACT busy), so every stage below is
    tuned to keep all three at their floor.
  - State kept transposed: h/c = [H=128 partitions, batch free]; cT = 2c so
    the g-gate tanh becomes a sigmoid: tanh(x) = 2*sigmoid(2x)-1, with the
    2x folded into the g-rows of the weights. tanh(c) itself is computed as
    Tanh(scale=0.5) on cT, so h = tanh(c)*sigma(o) is a plain multiply.
  - Gate order [i|f|g|o]: ONE sigmoid covers [i,f,g] (on the serial chain),
    a separate sigmoid covers o OFF the chain (o is first needed only after
    the tanh), shortening the chain by ~100ns/step.
  - Input + bias enter via ONE K=8 (phase P) / K=4 (phase H) matmul with a
    block-diagonal rhs (phase P rhs streamed from DRAM, phase H rhs static),
    accumulated into the gates PSUM bank before the 4 recurrent matmuls.
  - Cell update on DVE only (gpsimd is pathologically slow for elementwise):
      t2 = sf*cT;  u = (s2g-0.5)*si;  cT = 4u + t2;  h = tanh(cT/2)*so.
  - fp16 everywhere on-chip (not bf16): the 2*sigmoid(2x)-1 rewrite loses
    absolute precision near 0.5 in bf16; fp16's 10 mantissa bits restore it,
    and fp16 keeps the DVE 2x packed modes.
  - Predictions p_t = W_out h_t (+ b_out on host): h kept in an 8-slot ring;
    one shifted-stationary matmul per 4 steps per chunk accumulates 128
    steps into one PSUM bank, flushed to DRAM per 128-step epoch. The 8-slot
    ring lets the pred matmul run 1-2 steps late, emitted into a PE-queue
    slot measured to be stall-free (the in-order PE queue head-blocks on
    any instruction whose inputs are not ready yet).
  - The PE runs at 1.2 GHz on this part no matter what (sustained dense
    matmul bursts never unthrottle it), so matmul costs are budgeted cold.
"""

import functools

import numpy as np

import concourse.bacc as bacc
import concourse.tile as tile
from concourse import mybir
from concourse.bass_utils import run_bass_kernel_spmd

F32 = mybir.dt.float32
FP16 = mybir.dt.float16
AF = mybir.ActivationFunctionType
OP = mybir.AluOpType

H = 128          # hidden
NCORES = 8
BS = 256         # batch per core
BC = 128         # batch per chunk
NCHUNK = 2
BLK = 32         # xq steps per DMA block

# kernel gate order == pytorch order (i, f, g, o): sigma(i,f,g) is one
# contiguous on-chain activation; sigma(o) is separate and off-chain (o is
# first needed only after tanh(c)).
_PERM = np.arange(512)
# g-gate rows doubled so the matmul emits 2*pre_g for the
# tanh(x) = 2*sigmoid(2x)-1 rewrite; cT state = 2c.
_SCALE = np.repeat([1.0, 1.0, 2.0, 1.0], 128)
_SCALE_B = _SCALE


def _build_body(tc, d, NP, NH):
    nc = tc.nc
    NT = NP + NH
    NBLK = (NP + BLK - 1) // BLK

    import contextlib
    with contextlib.ExitStack() as ctx:
        consts = ctx.enter_context(tc.tile_pool(name="consts", bufs=1))
        state = ctx.enter_context(tc.tile_pool(name="state", bufs=1))
        spool = ctx.enter_context(tc.tile_pool(name="sig", bufs=3))
        wpool = ctx.enter_context(tc.tile_pool(name="work", bufs=3))
        xpool = ctx.enter_context(tc.tile_pool(name="xq", bufs=2))
        gpool = ctx.enter_context(tc.tile_pool(name="gates", bufs=2, space="PSUM"))
        opool = ctx.enter_context(tc.tile_pool(name="ogate", bufs=1, space="PSUM"))
        ppool = ctx.enter_context(tc.tile_pool(name="ppsum", bufs=1, space="PSUM"))

        # ---- constants to SBUF
        whhT_p = consts.tile([H, 4 * H], FP16, tag="whhT_p")
        whhT_h = consts.tile([H, 4 * H], FP16, tag="whhT_h")
        bp6 = consts.tile([6, H], FP16, tag="bp6")
        bpo = consts.tile([1, H], FP16, tag="bpo")
        bh4 = consts.tile([4, H], FP16, tag="bh4")
        ones4 = consts.tile([4, 4 * BC], FP16, tag="ones4")
        woutZ = consts.tile([H, 2 * H], FP16, tag="woutZ")
        bo_p = consts.tile([H, 1], F32, tag="bo_p")
        bo_h = consts.tile([H, 1], F32, tag="bo_h")
        nc.sync.dma_start(out=whhT_p, in_=d["whhT_p"])
        nc.sync.dma_start(out=whhT_h, in_=d["whhT_h"])
        nc.sync.dma_start(out=bp6, in_=d["bp6"])
        nc.sync.dma_start(out=bpo, in_=d["bpo"])
        nc.sync.dma_start(out=bh4, in_=d["bh4"])
        nc.sync.dma_start(out=ones4, in_=d["ones4"])
        nc.sync.dma_start(out=woutZ, in_=d["woutZ"])
        nc.sync.dma_start(out=bo_p, in_=d["bo_p"])
        nc.sync.dma_start(out=bo_h, in_=d["bo_h"])

        # ---- state
        hist = []
        cT = []
        for ch in range(NCHUNK):
            hh = state.tile([H, 8 * BC], FP16, tag=f"hist{ch}")
            c = state.tile([H, BC], FP16, tag=f"cT{ch}")
            nc.vector.memset(hh, 0.0)
            nc.vector.memset(c, 0.0)
            hist.append(hh)
            cT.append(c)

        # ---- xq streams (phase P block-diag rhs), double buffered
        xftiles = [[None] * NBLK for _ in range(NCHUNK)]
        xotiles = [[None] * NBLK for _ in range(NCHUNK)]

        def fetch(blk):
            for ch in range(NCHUNK):
                xf = xpool.tile([6, BLK * 3 * BC], FP16, tag=f"xqf{ch}",
                                name=f"xqf{ch}_{blk}")
                nc.sync.dma_start(out=xf, in_=d["xqf"][ch, blk])
                xftiles[ch][blk] = xf
                xo = xpool.tile([1, BLK * BC], FP16, tag=f"xqo{ch}",
                                name=f"xqo{ch}_{blk}")
                nc.sync.dma_start(out=xo, in_=d["xqo"][ch, blk])
                xotiles[ch][blk] = xo

        fetch(0)
        fetch(1)

        s4s = [None, None]
        sos = [None, None]
        pps = [None, None]

        def front(s, ch):
            """gate MMs + sigmoids.  The o-gate lives in its OWN psum bank:
            sigma(i,f,g) then only waits 3 recurrent MMs (psum bank-level
            serialization would otherwise make it wait the o MM too), the
            o-gate bias rides on sigma_o's per-partition bias AP, and the
            block-diagonal bias/input stream shrinks to 384 columns."""
            phase_p = s < NP
            gates = gpool.tile([H, 3 * BC], F32, tag=f"g{ch}",
                               name=f"g{ch}_{s}")
            og = opool.tile([H, BC], F32, tag=f"o{ch}", name=f"o{ch}_{s}")
            if phase_p:
                blk, sl = divmod(s, BLK)
                nc.tensor.matmul(gates, bp6,
                                 xftiles[ch][blk][:, sl * 3 * BC:
                                                  (sl + 1) * 3 * BC],
                                 start=True, stop=False,
                                 skip_group_check=True)
                nc.tensor.matmul(og, bpo,
                                 xotiles[ch][blk][:, sl * BC:(sl + 1) * BC],
                                 start=True, stop=False,
                                 skip_group_check=True)
            else:
                nc.tensor.matmul(gates, bh4[0:3, :], ones4[0:3, 0:3 * BC],
                                 start=True, stop=False,
                                 skip_group_check=True)
            whh = whhT_p if phase_p else whhT_h
            hprev = hist[ch][:, ((s - 1) % 8) * BC: ((s - 1) % 8 + 1) * BC]
            for j in range(3):
                nc.tensor.matmul(gates[:, j * H:(j + 1) * H],
                                 whh[:, j * H:(j + 1) * H], hprev,
                                 start=False, stop=(j == 2),
                                 skip_group_check=True)
            nc.tensor.matmul(og, whh[:, 3 * H:4 * H], hprev,
                             start=(not phase_p), stop=True,
                             skip_group_check=True)
            s4 = spool.tile([H, 3 * BC], FP16, tag=f"s4{ch}",
                            name=f"s4{ch}_{s}")
            nc.scalar.activation(s4, gates, AF.Sigmoid)
            so = spool.tile([H, BC], FP16, tag=f"so{ch}", name=f"so{ch}_{s}")
            nc.scalar.activation(so, og, AF.Sigmoid,
                                 bias=bo_p[:, 0:1] if phase_p
                                 else bo_h[:, 0:1])
            s4s[ch] = s4
            sos[ch] = so

        def back(s, ch):
            """cell update on DVE + sigma(cT) + hT + batched prediction MM."""
            s4 = s4s[ch]
            t2 = wpool.tile([H, BC], FP16, tag=f"t2{ch}", name=f"t2{ch}_{s}")
            nc.vector.tensor_mul(t2, s4[:, H:2 * H], cT[ch])
            u = wpool.tile([H, BC], FP16, tag=f"u{ch}", name=f"u{ch}_{s}")
            nc.vector.scalar_tensor_tensor(u, s4[:, 2 * H:3 * H], 0.5,
                                           s4[:, 0:H], OP.subtract, OP.mult)
            nc.vector.scalar_tensor_tensor(cT[ch], u, 4.0, t2,
                                           OP.mult, OP.add)
            tc_ = wpool.tile([H, BC], FP16, tag=f"sc{ch}", name=f"sc{ch}_{s}")
            nc.scalar.activation(tc_, cT[ch], AF.Tanh, scale=0.5)
            hslot = hist[ch][:, (s % 8) * BC: (s % 8 + 1) * BC]
            nc.vector.tensor_mul(hslot, tc_, sos[ch])


        def emit_pred(G, ch):
            NG = (NT + 3) // 4
            r = G % 32
            n = min(NT - 4 * G, 4) * BC
            base = (G % 2) * 4 * BC
            if r == 0:
                pps[ch] = ppool.tile([H, 4 * BC], F32, tag=f"pps{ch}",
                                     name=f"pps{ch}_{G}")
            nc.tensor.matmul(pps[ch][:, 0:n],
                             woutZ[:, H - r: 2 * H - r],
                             hist[ch][:, base: base + n],
                             start=(r == 0), stop=(r == 31 or G == NG - 1),
                             skip_group_check=True)
            if r == 31 or G == NG - 1:
                e = G // 32
                pc = wpool.tile([32, 4 * BC], F32, tag=f"pc{ch}",
                                name=f"pc{ch}_{G}")
                nc.vector.tensor_copy(pc, pps[ch][0:32, :])
                nc.sync.dma_start(out=d["preds"][e, ch], in_=pc)

        # Software pipeline: full A-step then full B-step per iteration.
        # Each engine's FIFO then alternates A-stage / B-stage, which locks
        # the two chunks half a step out of phase (emitting both fronts
        # together lets the chunks drift in-phase and exposes the full
        # serial chain latency).
        for s in range(NT):
            if s % BLK == BLK // 2:
                nb = s // BLK + 2
                if nb < NBLK:
                    fetch(nb)
            front(s, 0)
            back(s, 0)
            # Predictions (group G = steps 4G..4G+3) are emitted 1-2 steps
            # after the group completes, always in the PE-queue slot right
            # after chunk 0's gate MMs (measured stall-free: the pred matmul
            # is ready by then and fills the mid-step PE window).
            if s % 4 == 0 and s >= 4:
                emit_pred(s // 4 - 1, 0)
            if s % 4 == 2 and s >= 6:
                emit_pred((s - 2) // 4 - 1, 1)
            front(s, 1)
            back(s, 1)

        # final prediction group(s) not yet emitted by the loop
        NG = (NT + 3) // 4
        emit_pred(NG - 1, 0)
        emit_pred(NG - 1, 1)


@functools.lru_cache(maxsize=2)
def _program(NP, NH):
    nc = bacc.Bacc("TRN2", target_bir_lowering=False, debug=False,
                   num_devices=NCORES)
    NT = NP + NH
    NEP = (NT + 127) // 128
    NBLK = (NP + BLK - 1) // BLK
    d = {
        "whhT_p": nc.dram_tensor("whhT_p", [H, 4 * H], FP16,
                                 kind="ExternalInput").ap(),
        "whhT_h": nc.dram_tensor("whhT_h", [H, 4 * H], FP16,
                                 kind="ExternalInput").ap(),
        "bp6": nc.dram_tensor("bp6", [6, H], FP16, kind="ExternalInput").ap(),
        "bpo": nc.dram_tensor("bpo", [1, H], FP16, kind="ExternalInput").ap(),
        "bh4": nc.dram_tensor("bh4", [4, H], FP16, kind="ExternalInput").ap(),
        "ones4": nc.dram_tensor("ones4", [4, 4 * BC], FP16,
                                kind="ExternalInput").ap(),
        "woutZ": nc.dram_tensor("woutZ", [H, 2 * H], FP16,
                                kind="ExternalInput").ap(),
        "bo_p": nc.dram_tensor("bo_p", [H, 1], F32,
                               kind="ExternalInput").ap(),
        "bo_h": nc.dram_tensor("bo_h", [H, 1], F32,
                               kind="ExternalInput").ap(),
        "xqf": nc.dram_tensor("xqf", [NCHUNK, NBLK, 6, BLK * 3 * BC], FP16,
                              kind="ExternalInput").ap(),
        "xqo": nc.dram_tensor("xqo", [NCHUNK, NBLK, 1, BLK * BC], FP16,
                              kind="ExternalInput").ap(),
        "preds": nc.dram_tensor("preds", [NEP, NCHUNK, 32, 4 * BC], F32,
                                kind="ExternalOutput").ap(),
    }
    with tile.TileContext(nc) as tc:
        _build_body(tc, d, NP, NH)
    nc.compile()
    return nc


def _host_prep(y_flow, W_ih, W_hh, b_ih, b_hh, W_out, b_out, NP):
    """Build per-core input maps. y_flow: (B, T, 1) f32."""
    f16 = np.float16
    W_ih = np.asarray(W_ih, np.float32)
    W_hh = np.asarray(W_hh, np.float32)
    W_out = np.asarray(W_out, np.float32)
    bias = np.asarray(b_ih, np.float32) + np.asarray(b_hh, np.float32)
    b_out = np.asarray(b_out, np.float32)

    W_eff = W_hh + W_ih @ W_out           # [4H, H] (phase-H feedback fold)
    b_eff = bias + W_ih[:, 0] * b_out[0]

    sc = _SCALE[:, None]
    whhT_p = np.ascontiguousarray((W_hh[_PERM] * sc).T).astype(f16)
    whhT_h = np.ascontiguousarray((W_eff[_PERM] * sc).T).astype(f16)

    wih_s = (W_ih[_PERM, 0] * _SCALE_B).astype(np.float32)
    b_s = (bias[_PERM] * _SCALE_B).astype(np.float32)
    beff_s = (b_eff[_PERM] * _SCALE_B).astype(np.float32)

    bp6 = np.zeros((6, H), np.float32)
    bpo = wih_s[3 * H:4 * H].reshape(1, H).astype(np.float32)
    bh4 = np.zeros((4, H), np.float32)
    ones4 = np.zeros((4, 4 * BC), np.float32)
    for j in range(4):
        if j < 3:
            bp6[2 * j] = wih_s[j * H:(j + 1) * H]
            bp6[2 * j + 1] = b_s[j * H:(j + 1) * H]
        bh4[j] = beff_s[j * H:(j + 1) * H]
        ones4[j, j * BC:(j + 1) * BC] = 1.0

    woutZ = np.zeros((H, 2 * H), np.float32)
    woutZ[:, H] = W_out[0]
    bo_p = b_s[3 * H:4 * H].reshape(H, 1).astype(np.float32)
    bo_h = beff_s[3 * H:4 * H].reshape(H, 1).astype(np.float32)

    NBLK = (NP + BLK - 1) // BLK
    NPAD = NBLK * BLK
    y = np.asarray(y_flow, np.float32)[:, :, 0]                   # [B, T]
    in_maps = []
    for core in range(NCORES):
        yc = y[core * BS:(core + 1) * BS]                         # [BS, T]
        xqf = np.zeros((NCHUNK, NPAD, 6, 3 * BC), np.float32)
        xqo = np.zeros((NCHUNK, NPAD, 1, BC), np.float32)
        for ch in range(NCHUNK):
            ystep = yc[ch * BC:(ch + 1) * BC, :NP].T              # [NP, BC]
            for j in range(3):
                xqf[ch, :NP, 2 * j, j * BC:(j + 1) * BC] = ystep
                xqf[ch, :, 2 * j + 1, j * BC:(j + 1) * BC] = 1.0
            xqo[ch, :NP, 0, :] = ystep
        xqf = xqf.reshape(NCHUNK, NBLK, BLK, 6, 3 * BC)
        xqf = np.ascontiguousarray(xqf.transpose(0, 1, 3, 2, 4))
        xqf = xqf.reshape(NCHUNK, NBLK, 6, BLK * 3 * BC)
        xqo = xqo.reshape(NCHUNK, NBLK, BLK, 1, BC)
        xqo = np.ascontiguousarray(xqo.transpose(0, 1, 3, 2, 4))
        xqo = xqo.reshape(NCHUNK, NBLK, 1, BLK * BC)
        in_maps.append({
            "whhT_p": whhT_p, "whhT_h": whhT_h,
            "bp6": bp6.astype(f16), "bpo": bpo.astype(f16),
            "bh4": bh4.astype(f16),
            "ones4": ones4.astype(f16), "woutZ": woutZ.astype(f16),
            "bo_p": bo_p, "bo_h": bo_h,
            "xqf": xqf.astype(f16), "xqo": xqo.astype(f16),
        })
    return in_maps


def kernel(y_flow, x_dyn, W_ih, W_hh, b_ih, b_hh, W_out, b_out, twin_idx,
           _trace=False):
    twin = int(twin_idx)
    assert twin == 256, f"kernel hardcodes twin_idx=256, got {twin}"
    B, T, _ = y_flow.shape
    assert (B, T) == (2048, 512)
    NP, NH = twin - 1, T - twin
    NT = NP + NH

    nc = _program(NP, NH)
    in_maps = _host_prep(y_flow, W_ih, W_hh, b_ih, b_hh, W_out, b_out, NP)
    res = run_bass_kernel_spmd(nc, in_maps, core_ids=list(range(NCORES)),
                               trace=_trace)

    b_out = np.asarray(b_out, np.float32)
    out = np.empty((B, NT, 1), np.float32)
    for core in range(NCORES):
        p = np.asarray(res.results[core]["preds"], np.float32)
        nep = p.shape[0]
        a = p.reshape(nep, NCHUNK, 32, 4, BC)      # [e, ch, r, j, b]
        for ch in range(NCHUNK):
            blk = a[:, ch].transpose(3, 0, 1, 2).reshape(BC, -1)[:, :NT]
            out[core * BS + ch * BC: core * BS + (ch + 1) * BC, :, 0] = \
                blk + b_out[0]
    if _trace:
        kernel._last_results = res
    return out


# revision 41
# speedup vs baseline: 1.1978x; 1.1978x over previous
"""Bass/Trainium2 kernel for nn_BaselineLSTM (B=2048, T=512, H=128, twin=256).

Strategy (final):
  - Data-parallel: batch 2048 -> 8 cores x 256; each core runs 2 interleaved
    chunks of 128 batch. The kernel is bound three ways at ~2.9us/step
    (per-chunk serial chain ~ PE busy ~ ACT busy), so every stage below is
    tuned to keep all three at their floor.
  - State kept transposed: h/c = [H=128 partitions, batch free]; cT = 2c so
    the g-gate tanh becomes a sigmoid: tanh(x) = 2*sigmoid(2x)-1, with the
    2x folded into the g-rows of the weights. tanh(c) itself is computed as
    Tanh(scale=0.5) on cT, so h = tanh(c)*sigma(o) is a plain multiply.
  - Gate order [i|f|g|o]: ONE sigmoid covers [i,f,g] (on the serial chain),
    a separate sigmoid covers o OFF the chain (o is first needed only after
    the tanh), shortening the chain by ~100ns/step.
  - Input + bias enter via ONE K=8 (phase P) / K=4 (phase H) matmul with a
    block-diagonal rhs (phase P rhs streamed from DRAM, phase H rhs static),
    accumulated into the gates PSUM bank before the 4 recurrent matmuls.
  - Cell update on DVE only (gpsimd is pathologically slow for elementwise):
      t2 = sf*cT;  u = (s2g-0.5)*si;  cT = 4u + t2;  h = tanh(cT/2)*so.
  - fp16 everywhere on-chip (not bf16): the 2*sigmoid(2x)-1 rewrite loses
    absolute precision near 0.5 in bf16; fp16's 10 mantissa bits restore it,
    and fp16 keeps the DVE 2x packed modes.
  - Predictions p_t = W_out h_t (+ b_out on host): h kept in an 8-slot ring;
    one shifted-stationary matmul per 4 steps per chunk accumulates 128
    steps into one PSUM bank, flushed to DRAM per 128-step epoch. The 8-slot
    ring lets the pred matmul run 1-2 steps late, emitted into a PE-queue
    slot measured to be stall-free (the in-order PE queue head-blocks on
    any instruction whose inputs are not ready yet).
  - The PE runs at 1.2 GHz on this part no matter what (sustained dense
    matmul bursts never unthrottle it), so matmul costs are budgeted cold.
"""

import functools

import numpy as np

import concourse.bacc as bacc
import concourse.tile as tile
from concourse import mybir
from concourse.bass_utils import run_bass_kernel_spmd

F32 = mybir.dt.float32
FP16 = mybir.dt.float16
AF = mybir.ActivationFunctionType
OP = mybir.AluOpType

H = 128          # hidden
NCORES = 8
BS = 256         # batch per core
BC = 128         # batch per chunk
NCHUNK = 2
BLK = 32         # xq steps per DMA block

# kernel gate order == pytorch order (i, f, g, o): sigma(i,f,g) is one
# contiguous on-chain activation; sigma(o) is separate and off-chain (o is
# first needed only after tanh(c)).
_PERM = np.arange(512)
# g-gate rows doubled so the matmul emits 2*pre_g for the
# tanh(x) = 2*sigmoid(2x)-1 rewrite; cT state = 2c.
_SCALE = np.repeat([1.0, 1.0, 2.0, 1.0], 128)
_SCALE_B = _SCALE


def _build_body(tc, d, NP, NH):
    nc = tc.nc
    NT = NP + NH
    NBLK = (NP + BLK - 1) // BLK

    import contextlib
    with contextlib.ExitStack() as ctx:
        consts = ctx.enter_context(tc.tile_pool(name="consts", bufs=1))
        state = ctx.enter_context(tc.tile_pool(name="state", bufs=1))
        spool = ctx.enter_context(tc.tile_pool(name="sig", bufs=3))
        wpool = ctx.enter_context(tc.tile_pool(name="work", bufs=3))
        xpool = ctx.enter_context(tc.tile_pool(name="xq", bufs=2))
        gpool = ctx.enter_context(tc.tile_pool(name="gates", bufs=1, space="PSUM"))
        opool = ctx.enter_context(tc.tile_pool(name="ogate", bufs=2, space="PSUM"))
        ppool = ctx.enter_context(tc.tile_pool(name="ppsum", bufs=1, space="PSUM"))

        # ---- constants to SBUF
        whhT_p = consts.tile([H, 4 * H], FP16, tag="whhT_p")
        whhT_h = consts.tile([H, 4 * H], FP16, tag="whhT_h")
        bp6 = consts.tile([6, H], FP16, tag="bp6")
        bpo = consts.tile([1, H], FP16, tag="bpo")
        bh4 = consts.tile([4, H], FP16, tag="bh4")
        ones4 = consts.tile([4, 4 * BC], FP16, tag="ones4")
        woutZ = consts.tile([H, 2 * H], FP16, tag="woutZ")
        bo_p = consts.tile([H, 1], F32, tag="bo_p")
        bo_h = consts.tile([H, 1], F32, tag="bo_h")
        nc.sync.dma_start(out=whhT_p, in_=d["whhT_p"])
        nc.sync.dma_start(out=whhT_h, in_=d["whhT_h"])
        nc.sync.dma_start(out=bp6, in_=d["bp6"])
        nc.sync.dma_start(out=bpo, in_=d["bpo"])
        nc.sync.dma_start(out=bh4, in_=d["bh4"])
        nc.sync.dma_start(out=ones4, in_=d["ones4"])
        nc.sync.dma_start(out=woutZ, in_=d["woutZ"])
        nc.sync.dma_start(out=bo_p, in_=d["bo_p"])
        nc.sync.dma_start(out=bo_h, in_=d["bo_h"])

        # ---- state
        hist = []
        cT = []
        for ch in range(NCHUNK):
            hh = state.tile([H, 8 * BC], FP16, tag=f"hist{ch}")
            c = state.tile([H, BC], FP16, tag=f"cT{ch}")
            nc.vector.memset(hh, 0.0)
            nc.vector.memset(c, 0.0)
            hist.append(hh)
            cT.append(c)

        # ---- xq streams (phase P block-diag rhs), double buffered
        xftiles = [[None] * NBLK for _ in range(NCHUNK)]
        xotiles = [[None] * NBLK for _ in range(NCHUNK)]

        def fetch(blk):
            for ch in range(NCHUNK):
                xf = xpool.tile([6, BLK * 3 * BC], FP16, tag=f"xqf{ch}",
                                name=f"xqf{ch}_{blk}")
                nc.sync.dma_start(out=xf, in_=d["xqf"][ch, blk])
                xftiles[ch][blk] = xf
                xo = xpool.tile([1, BLK * BC], FP16, tag=f"xqo{ch}",
                                name=f"xqo{ch}_{blk}")
                nc.sync.dma_start(out=xo, in_=d["xqo"][ch, blk])
                xotiles[ch][blk] = xo

        fetch(0)
        fetch(1)

        s4s = [None, None]
        sos = [None, None]
        pps = [None, None]

        def front(s, ch):
            """gate MMs + sigmoids.  The o-gate lives in its OWN psum bank:
            sigma(i,f,g) then only waits 3 recurrent MMs (psum bank-level
            serialization would otherwise make it wait the o MM too), the
            o-gate bias rides on sigma_o's per-partition bias AP, and the
            block-diagonal bias/input stream shrinks to 384 columns."""
            phase_p = s < NP
            gates = gpool.tile([H, 3 * BC], F32, tag=f"g{ch}",
                               name=f"g{ch}_{s}")
            og = opool.tile([H, BC], F32, tag=f"o{ch}", name=f"o{ch}_{s}")
            if phase_p:
                blk, sl = divmod(s, BLK)
                nc.tensor.matmul(gates, bp6,
                                 xftiles[ch][blk][:, sl * 3 * BC:
                                                  (sl + 1) * 3 * BC],
                                 start=True, stop=False,
                                 skip_group_check=True)
                nc.tensor.matmul(og, bpo,
                                 xotiles[ch][blk][:, sl * BC:(sl + 1) * BC],
                                 start=True, stop=False,
                                 skip_group_check=True)
            else:
                nc.tensor.matmul(gates, bh4[0:3, :], ones4[0:3, 0:3 * BC],
                                 start=True, stop=False,
                                 skip_group_check=True)
            whh = whhT_p if phase_p else whhT_h
            hprev = hist[ch][:, ((s - 1) % 8) * BC: ((s - 1) % 8 + 1) * BC]
            for j in range(3):
                nc.tensor.matmul(gates[:, j * H:(j + 1) * H],
                                 whh[:, j * H:(j + 1) * H], hprev,
                                 start=False, stop=(j == 2),
                                 skip_group_check=True)
            nc.tensor.matmul(og, whh[:, 3 * H:4 * H], hprev,
                             start=(not phase_p), stop=True,
                             skip_group_check=True)
            s4 = spool.tile([H, 3 * BC], FP16, tag=f"s4{ch}",
                            name=f"s4{ch}_{s}")
            nc.scalar.activation(s4, gates, AF.Sigmoid)
            so = spool.tile([H, BC], FP16, tag=f"so{ch}", name=f"so{ch}_{s}")
            nc.scalar.activation(so, og, AF.Sigmoid,
                                 bias=bo_p[:, 0:1] if phase_p
                                 else bo_h[:, 0:1])
            s4s[ch] = s4
            sos[ch] = so

        def back(s, ch):
            """cell update on DVE + sigma(cT) + hT + batched prediction MM."""
            s4 = s4s[ch]
            t2 = wpool.tile([H, BC], FP16, tag=f"t2{ch}", name=f"t2{ch}_{s}")
            nc.vector.tensor_mul(t2, s4[:, H:2 * H], cT[ch])
            u = wpool.tile([H, BC], FP16, tag=f"u{ch}", name=f"u{ch}_{s}")
            nc.vector.scalar_tensor_tensor(u, s4[:, 2 * H:3 * H], 0.5,
                                           s4[:, 0:H], OP.subtract, OP.mult)
            nc.vector.scalar_tensor_tensor(cT[ch], u, 4.0, t2,
                                           OP.mult, OP.add)
            tc_ = wpool.tile([H, BC], FP16, tag=f"sc{ch}", name=f"sc{ch}_{s}")
            nc.scalar.activation(tc_, cT[ch], AF.Tanh, scale=0.5)
            hslot = hist[ch][:, (s % 8) * BC: (s % 8 + 1) * BC]
            nc.vector.tensor_mul(hslot, tc_, sos[ch])


        def emit_pred(G, ch):
            NG = (NT + 3) // 4
            r = G % 32
            n = min(NT - 4 * G, 4) * BC
            base = (G % 2) * 4 * BC
            if r == 0:
                pps[ch] = ppool.tile([H, 4 * BC], F32, tag=f"pps{ch}",
                                     name=f"pps{ch}_{G}")
            nc.tensor.matmul(pps[ch][:, 0:n],
                             woutZ[:, H - r: 2 * H - r],
                             hist[ch][:, base: base + n],
                             start=(r == 0), stop=(r == 31 or G == NG - 1),
                             skip_group_check=True)
            if r == 31 or G == NG - 1:
                e = G // 32
                pc = wpool.tile([32, 4 * BC], F32, tag=f"pc{ch}",
                                name=f"pc{ch}_{G}")
                nc.vector.tensor_copy(pc, pps[ch][0:32, :])
                nc.sync.dma_start(out=d["preds"][e, ch], in_=pc)

        # Software pipeline: full A-step then full B-step per iteration.
        # Each engine's FIFO then alternates A-stage / B-stage, which locks
        # the two chunks half a step out of phase (emitting both fronts
        # together lets the chunks drift in-phase and exposes the full
        # serial chain latency).
        for s in range(NT):
            if s % BLK == BLK // 2:
                nb = s // BLK + 2
                if nb < NBLK:
                    fetch(nb)
            front(s, 0)
            back(s, 0)
            # Predictions (group G = steps 4G..4G+3) are emitted 1-2 steps
            # after the group completes, always in the PE-queue slot right
            # after chunk 0's gate MMs (measured stall-free: the pred matmul
            # is ready by then and fills the mid-step PE window).
            if s % 4 == 0 and s >= 4:
                emit_pred(s // 4 - 1, 0)
            if s % 4 == 2 and s >= 6:
                emit_pred((s - 2) // 4 - 1, 1)
            front(s, 1)
            back(s, 1)

        # final prediction group(s) not yet emitted by the loop
        NG = (NT + 3) // 4
        emit_pred(NG - 1, 0)
        emit_pred(NG - 1, 1)


@functools.lru_cache(maxsize=2)
def _program(NP, NH):
    nc = bacc.Bacc("TRN2", target_bir_lowering=False, debug=False,
                   num_devices=NCORES)
    NT = NP + NH
    NEP = (NT + 127) // 128
    NBLK = (NP + BLK - 1) // BLK
    d = {
        "whhT_p": nc.dram_tensor("whhT_p", [H, 4 * H], FP16,
                                 kind="ExternalInput").ap(),
        "whhT_h": nc.dram_tensor("whhT_h", [H, 4 * H], FP16,
                                 kind="ExternalInput").ap(),
        "bp6": nc.dram_tensor("bp6", [6, H], FP16, kind="ExternalInput").ap(),
        "bpo": nc.dram_tensor("bpo", [1, H], FP16, kind="ExternalInput").ap(),
        "bh4": nc.dram_tensor("bh4", [4, H], FP16, kind="ExternalInput").ap(),
        "ones4": nc.dram_tensor("ones4", [4, 4 * BC], FP16,
                                kind="ExternalInput").ap(),
        "woutZ": nc.dram_tensor("woutZ", [H, 2 * H], FP16,
                                kind="ExternalInput").ap(),
        "bo_p": nc.dram_tensor("bo_p", [H, 1], F32,
                               kind="ExternalInput").ap(),
        "bo_h": nc.dram_tensor("bo_h", [H, 1], F32,
                               kind="ExternalInput").ap(),
        "xqf": nc.dram_tensor("xqf", [NCHUNK, NBLK, 6, BLK * 3 * BC], FP16,
                              kind="ExternalInput").ap(),
        "xqo": nc.dram_tensor("xqo", [NCHUNK, NBLK, 1, BLK * BC], FP16,
                              kind="ExternalInput").ap(),
        "preds": nc.dram_tensor("preds", [NEP, NCHUNK, 32, 4 * BC], F32,
                                kind="ExternalOutput").ap(),
    }
    with tile.TileContext(nc) as tc:
        _build_body(tc, d, NP, NH)
    nc.compile()
    return nc


def _host_prep(y_flow, W_ih, W_hh, b_ih, b_hh, W_out, b_out, NP):
    """Build per-core input maps. y_flow: (B, T, 1) f32."""
    f16 = np.float16
    W_ih = np.asarray(W_ih, np.float32)
    W_hh = np.asarray(W_hh, np.float32)
    W_out = np.asarray(W_out, np.float32)
    bias = np.asarray(b_ih, np.float32) + np.asarray(b_hh, np.float32)
    b_out = np.asarray(b_out, np.float32)

    W_eff = W_hh + W_ih @ W_out           # [4H, H] (phase-H feedback fold)
    b_eff = bias + W_ih[:, 0] * b_out[0]

    sc = _SCALE[:, None]
    whhT_p = np.ascontiguousarray((W_hh[_PERM] * sc).T).astype(f16)
    whhT_h = np.ascontiguousarray((W_eff[_PERM] * sc).T).astype(f16)

    wih_s = (W_ih[_PERM, 0] * _SCALE_B).astype(np.float32)
    b_s = (bias[_PERM] * _SCALE_B).astype(np.float32)
    beff_s = (b_eff[_PERM] * _SCALE_B).astype(np.float32)

    bp6 = np.zeros((6, H), np.float32)
    bpo = wih_s[3 * H:4 * H].reshape(1, H).astype(np.float32)
    bh4 = np.zeros((4, H), np.float32)
    ones4 = np.zeros((4, 4 * BC), np.float32)
    for j in range(4):
        if j < 3:
            bp6[2 * j] = wih_s[j * H:(j + 1) * H]
            bp6[2 * j + 1] = b_s[j * H:(j + 1) * H]
        bh4[j] = beff_s[j * H:(j + 1) * H]
        ones4[j, j * BC:(j + 1) * BC] = 1.0

    woutZ = np.zeros((H, 2 * H), np.float32)
    woutZ[:, H] = W_out[0]
    bo_p = b_s[3 * H:4 * H].reshape(H, 1).astype(np.float32)
    bo_h = beff_s[3 * H:4 * H].reshape(H, 1).astype(np.float32)

    NBLK = (NP + BLK - 1) // BLK
    NPAD = NBLK * BLK
    y = np.asarray(y_flow, np.float32)[:, :, 0]                   # [B, T]
    in_maps = []
    for core in range(NCORES):
        yc = y[core * BS:(core + 1) * BS]                         # [BS, T]
        xqf = np.zeros((NCHUNK, NPAD, 6, 3 * BC), np.float32)
        xqo = np.zeros((NCHUNK, NPAD, 1, BC), np.float32)
        for ch in range(NCHUNK):
            ystep = yc[ch * BC:(ch + 1) * BC, :NP].T              # [NP, BC]
            for j in range(3):
                xqf[ch, :NP, 2 * j, j * BC:(j + 1) * BC] = ystep
                xqf[ch, :, 2 * j + 1, j * BC:(j + 1) * BC] = 1.0
            xqo[ch, :NP, 0, :] = ystep
        xqf = xqf.reshape(NCHUNK, NBLK, BLK, 6, 3 * BC)
        xqf = np.ascontiguousarray(xqf.transpose(0, 1, 3, 2, 4))
        xqf = xqf.reshape(NCHUNK, NBLK, 6, BLK * 3 * BC)
        xqo = xqo.reshape(NCHUNK, NBLK, BLK, 1, BC)
        xqo = np.ascontiguousarray(xqo.transpose(0, 1, 3, 2, 4))
        xqo = xqo.reshape(NCHUNK, NBLK, 1, BLK * BC)
        in_maps.append({
            "whhT_p": whhT_p, "whhT_h": whhT_h,
            "bp6": bp6.astype(f16), "bpo": bpo.astype(f16),
            "bh4": bh4.astype(f16),
            "ones4": ones4.astype(f16), "woutZ": woutZ.astype(f16),
            "bo_p": bo_p, "bo_h": bo_h,
            "xqf": xqf.astype(f16), "xqo": xqo.astype(f16),
        })
    return in_maps


def kernel(y_flow, x_dyn, W_ih, W_hh, b_ih, b_hh, W_out, b_out, twin_idx,
           _trace=False):
    twin = int(twin_idx)
    assert twin == 256, f"kernel hardcodes twin_idx=256, got {twin}"
    B, T, _ = y_flow.shape
    assert (B, T) == (2048, 512)
    NP, NH = twin - 1, T - twin
    NT = NP + NH

    nc = _program(NP, NH)
    in_maps = _host_prep(y_flow, W_ih, W_hh, b_ih, b_hh, W_out, b_out, NP)
    res = run_bass_kernel_spmd(nc, in_maps, core_ids=list(range(NCORES)),
                               trace=_trace)

    b_out = np.asarray(b_out, np.float32)
    out = np.empty((B, NT, 1), np.float32)
    for core in range(NCORES):
        p = np.asarray(res.results[core]["preds"], np.float32)
        nep = p.shape[0]
        a = p.reshape(nep, NCHUNK, 32, 4, BC)      # [e, ch, r, j, b]
        for ch in range(NCHUNK):
            blk = a[:, ch].transpose(3, 0, 1, 2).reshape(BC, -1)[:, :NT]
            out[core * BS + ch * BC: core * BS + (ch + 1) * BC, :, 0] = \
                blk + b_out[0]
    if _trace:
        kernel._last_results = res
    return out


# revision 42
# speedup vs baseline: 1.1991x; 1.0011x over previous
"""Bass/Trainium2 kernel for nn_BaselineLSTM (B=2048, T=512, H=128, twin=256).

Strategy (final):
  - Data-parallel: batch 2048 -> 8 cores x 256; each core runs 2 interleaved
    chunks of 128 batch. The kernel is bound three ways at ~2.9us/step
    (per-chunk serial chain ~ PE busy ~ ACT busy), so every stage below is
    tuned to keep all three at their floor.
  - State kept transposed: h/c = [H=128 partitions, batch free]; cT = 2c so
    the g-gate tanh becomes a sigmoid: tanh(x) = 2*sigmoid(2x)-1, with the
    2x folded into the g-rows of the weights. tanh(c) itself is computed as
    Tanh(scale=0.5) on cT, so h = tanh(c)*sigma(o) is a plain multiply.
  - Gate order [i|f|g|o]: ONE sigmoid covers [i,f,g] (on the serial chain),
    a separate sigmoid covers o OFF the chain (o is first needed only after
    the tanh), shortening the chain by ~100ns/step.
  - Input + bias enter via ONE K=8 (phase P) / K=4 (phase H) matmul with a
    block-diagonal rhs (phase P rhs streamed from DRAM, phase H rhs static),
    accumulated into the gates PSUM bank before the 4 recurrent matmuls.
  - Cell update on DVE only (gpsimd is pathologically slow for elementwise):
      t2 = sf*cT;  u = (s2g-0.5)*si;  cT = 4u + t2;  h = tanh(cT/2)*so.
  - fp16 everywhere on-chip (not bf16): the 2*sigmoid(2x)-1 rewrite loses
    absolute precision near 0.5 in bf16; fp16's 10 mantissa bits restore it,
    and fp16 keeps the DVE 2x packed modes.
  - Predictions p_t = W_out h_t (+ b_out on host): h kept in an 8-slot ring;
    one shifted-stationary matmul per 4 steps per chunk accumulates 128
    steps into one PSUM bank, flushed to DRAM per 128-step epoch. The 8-slot
    ring lets the pred matmul run 1-2 steps late, emitted into a PE-queue
    slot measured to be stall-free (the in-order PE queue head-blocks on
    any instruction whose inputs are not ready yet).
  - The PE runs at 1.2 GHz on this part no matter what (sustained dense
    matmul bursts never unthrottle it), so matmul costs are budgeted cold.
"""

import functools

import numpy as np

import concourse.bacc as bacc
import concourse.tile as tile
from concourse import mybir
from concourse.bass_utils import run_bass_kernel_spmd

F32 = mybir.dt.float32
FP16 = mybir.dt.float16
AF = mybir.ActivationFunctionType
OP = mybir.AluOpType

H = 128          # hidden
NCORES = 8
BS = 256         # batch per core
BC = 128         # batch per chunk
NCHUNK = 2
BLK = 32         # xq steps per DMA block

# kernel gate order == pytorch order (i, f, g, o): sigma(i,f,g) is one
# contiguous on-chain activation; sigma(o) is separate and off-chain (o is
# first needed only after tanh(c)).
_PERM = np.arange(512)
# g-gate rows doubled so the matmul emits 2*pre_g for the
# tanh(x) = 2*sigmoid(2x)-1 rewrite; cT state = 2c.
_SCALE = np.repeat([1.0, 1.0, 2.0, 1.0], 128)
_SCALE_B = _SCALE


def _build_body(tc, d, NP, NH):
    nc = tc.nc
    NT = NP + NH
    NBLK = (NP + BLK - 1) // BLK

    import contextlib
    with contextlib.ExitStack() as ctx:
        consts = ctx.enter_context(tc.tile_pool(name="consts", bufs=1))
        state = ctx.enter_context(tc.tile_pool(name="state", bufs=1))
        spool = ctx.enter_context(tc.tile_pool(name="sig", bufs=4))
        wpool = ctx.enter_context(tc.tile_pool(name="work", bufs=4))
        xpool = ctx.enter_context(tc.tile_pool(name="xq", bufs=2))
        gpool = ctx.enter_context(tc.tile_pool(name="gates", bufs=1, space="PSUM"))
        opool = ctx.enter_context(tc.tile_pool(name="ogate", bufs=2, space="PSUM"))
        ppool = ctx.enter_context(tc.tile_pool(name="ppsum", bufs=1, space="PSUM"))

        # ---- constants to SBUF
        whhT_p = consts.tile([H, 4 * H], FP16, tag="whhT_p")
        whhT_h = consts.tile([H, 4 * H], FP16, tag="whhT_h")
        bp6 = consts.tile([6, H], FP16, tag="bp6")
        bpo = consts.tile([1, H], FP16, tag="bpo")
        bh4 = consts.tile([4, H], FP16, tag="bh4")
        ones4 = consts.tile([4, 4 * BC], FP16, tag="ones4")
        woutZ = consts.tile([H, 2 * H], FP16, tag="woutZ")
        bo_p = consts.tile([H, 1], F32, tag="bo_p")
        bo_h = consts.tile([H, 1], F32, tag="bo_h")
        nc.sync.dma_start(out=whhT_p, in_=d["whhT_p"])
        nc.sync.dma_start(out=whhT_h, in_=d["whhT_h"])
        nc.sync.dma_start(out=bp6, in_=d["bp6"])
        nc.sync.dma_start(out=bpo, in_=d["bpo"])
        nc.sync.dma_start(out=bh4, in_=d["bh4"])
        nc.sync.dma_start(out=ones4, in_=d["ones4"])
        nc.sync.dma_start(out=woutZ, in_=d["woutZ"])
        nc.sync.dma_start(out=bo_p, in_=d["bo_p"])
        nc.sync.dma_start(out=bo_h, in_=d["bo_h"])

        # ---- state
        hist = []
        cT = []
        for ch in range(NCHUNK):
            hh = state.tile([H, 8 * BC], FP16, tag=f"hist{ch}")
            c = state.tile([H, BC], FP16, tag=f"cT{ch}")
            nc.vector.memset(hh, 0.0)
            nc.vector.memset(c, 0.0)
            hist.append(hh)
            cT.append(c)

        # ---- xq streams (phase P block-diag rhs), double buffered
        xftiles = [[None] * NBLK for _ in range(NCHUNK)]
        xotiles = [[None] * NBLK for _ in range(NCHUNK)]

        def fetch(blk):
            for ch in range(NCHUNK):
                xf = xpool.tile([6, BLK * 3 * BC], FP16, tag=f"xqf{ch}",
                                name=f"xqf{ch}_{blk}")
                nc.sync.dma_start(out=xf, in_=d["xqf"][ch, blk])
                xftiles[ch][blk] = xf
                xo = xpool.tile([1, BLK * BC], FP16, tag=f"xqo{ch}",
                                name=f"xqo{ch}_{blk}")
                nc.sync.dma_start(out=xo, in_=d["xqo"][ch, blk])
                xotiles[ch][blk] = xo

        fetch(0)
        fetch(1)

        s4s = [None, None]
        sos = [None, None]
        pps = [None, None]

        def front(s, ch):
            """gate MMs + sigmoids.  The o-gate lives in its OWN psum bank:
            sigma(i,f,g) then only waits 3 recurrent MMs (psum bank-level
            serialization would otherwise make it wait the o MM too), the
            o-gate bias rides on sigma_o's per-partition bias AP, and the
            block-diagonal bias/input stream shrinks to 384 columns."""
            phase_p = s < NP
            gates = gpool.tile([H, 3 * BC], F32, tag=f"g{ch}",
                               name=f"g{ch}_{s}")
            og = opool.tile([H, BC], F32, tag=f"o{ch}", name=f"o{ch}_{s}")
            if phase_p:
                blk, sl = divmod(s, BLK)
                nc.tensor.matmul(gates, bp6,
                                 xftiles[ch][blk][:, sl * 3 * BC:
                                                  (sl + 1) * 3 * BC],
                                 start=True, stop=False,
                                 skip_group_check=True)
                nc.tensor.matmul(og, bpo,
                                 xotiles[ch][blk][:, sl * BC:(sl + 1) * BC],
                                 start=True, stop=False,
                                 skip_group_check=True)
            else:
                nc.tensor.matmul(gates, bh4[0:3, :], ones4[0:3, 0:3 * BC],
                                 start=True, stop=False,
                                 skip_group_check=True)
            whh = whhT_p if phase_p else whhT_h
            hprev = hist[ch][:, ((s - 1) % 8) * BC: ((s - 1) % 8 + 1) * BC]
            for j in range(3):
                nc.tensor.matmul(gates[:, j * H:(j + 1) * H],
                                 whh[:, j * H:(j + 1) * H], hprev,
                                 start=False, stop=(j == 2),
                                 skip_group_check=True)
            nc.tensor.matmul(og, whh[:, 3 * H:4 * H], hprev,
                             start=(not phase_p), stop=True,
                             skip_group_check=True)
            s4 = spool.tile([H, 3 * BC], FP16, tag=f"s4{ch}",
                            name=f"s4{ch}_{s}")
            nc.scalar.activation(s4, gates, AF.Sigmoid)
            so = spool.tile([H, BC], FP16, tag=f"so{ch}", name=f"so{ch}_{s}")
            nc.scalar.activation(so, og, AF.Sigmoid,
                                 bias=bo_p[:, 0:1] if phase_p
                                 else bo_h[:, 0:1])
            s4s[ch] = s4
            sos[ch] = so

        def back(s, ch):
            """cell update on DVE + sigma(cT) + hT + batched prediction MM."""
            s4 = s4s[ch]
            t2 = wpool.tile([H, BC], FP16, tag=f"t2{ch}", name=f"t2{ch}_{s}")
            nc.vector.tensor_mul(t2, s4[:, H:2 * H], cT[ch])
            u = wpool.tile([H, BC], FP16, tag=f"u{ch}", name=f"u{ch}_{s}")
            nc.vector.scalar_tensor_tensor(u, s4[:, 2 * H:3 * H], 0.5,
                                           s4[:, 0:H], OP.subtract, OP.mult)
            nc.vector.scalar_tensor_tensor(cT[ch], u, 4.0, t2,
                                           OP.mult, OP.add)
            tc_ = wpool.tile([H, BC], FP16, tag=f"sc{ch}", name=f"sc{ch}_{s}")
            nc.scalar.activation(tc_, cT[ch], AF.Tanh, scale=0.5)
            hslot = hist[ch][:, (s % 8) * BC: (s % 8 + 1) * BC]
            nc.vector.tensor_mul(hslot, tc_, sos[ch])


        def emit_pred(G, ch):
            NG = (NT + 3) // 4
            r = G % 32
            n = min(NT - 4 * G, 4) * BC
            base = (G % 2) * 4 * BC
            if r == 0:
                pps[ch] = ppool.tile([H, 4 * BC], F32, tag=f"pps{ch}",
                                     name=f"pps{ch}_{G}")
            nc.tensor.matmul(pps[ch][:, 0:n],
                             woutZ[:, H - r: 2 * H - r],
                             hist[ch][:, base: base + n],
                             start=(r == 0), stop=(r == 31 or G == NG - 1),
                             skip_group_check=True)
            if r == 31 or G == NG - 1:
                e = G // 32
                pc = wpool.tile([32, 4 * BC], F32, tag=f"pc{ch}",
                                name=f"pc{ch}_{G}")
                nc.vector.tensor_copy(pc, pps[ch][0:32, :])
                nc.sync.dma_start(out=d["preds"][e, ch], in_=pc)

        # Software pipeline: full A-step then full B-step per iteration.
        # Each engine's FIFO then alternates A-stage / B-stage, which locks
        # the two chunks half a step out of phase (emitting both fronts
        # together lets the chunks drift in-phase and exposes the full
        # serial chain latency).
        for s in range(NT):
            if s % BLK == BLK // 2:
                nb = s // BLK + 2
                if nb < NBLK:
                    fetch(nb)
            front(s, 0)
            back(s, 0)
            # Predictions (group G = steps 4G..4G+3) are emitted 1-2 steps
            # after the group completes, always in the PE-queue slot right
            # after chunk 0's gate MMs (measured stall-free: the pred matmul
            # is ready by then and fills the mid-step PE window).
            if s % 4 == 0 and s >= 4:
                emit_pred(s // 4 - 1, 0)
            if s % 4 == 2 and s >= 6:
                emit_pred((s - 2) // 4 - 1, 1)
            front(s, 1)
            back(s, 1)

        # final prediction group(s) not yet emitted by the loop
        NG = (NT + 3) // 4
        emit_pred(NG - 1, 0)
        emit_pred(NG - 1, 1)


@functools.lru_cache(maxsize=2)
def _program(NP, NH):
    nc = bacc.Bacc("TRN2", target_bir_lowering=False, debug=False,
                   num_devices=NCORES)
    NT = NP + NH
    NEP = (NT + 127) // 128
    NBLK = (NP + BLK - 1) // BLK
    d = {
        "whhT_p": nc.dram_tensor("whhT_p", [H, 4 * H], FP16,
                                 kind="ExternalInput").ap(),
        "whhT_h": nc.dram_tensor("whhT_h", [H, 4 * H], FP16,
                                 kind="ExternalInput").ap(),
        "bp6": nc.dram_tensor("bp6", [6, H], FP16, kind="ExternalInput").ap(),
        "bpo": nc.dram_tensor("bpo", [1, H], FP16, kind="ExternalInput").ap(),
        "bh4": nc.dram_tensor("bh4", [4, H], FP16, kind="ExternalInput").ap(),
        "ones4": nc.dram_tensor("ones4", [4, 4 * BC], FP16,
                                kind="ExternalInput").ap(),
        "woutZ": nc.dram_tensor("woutZ", [H, 2 * H], FP16,
                                kind="ExternalInput").ap(),
        "bo_p": nc.dram_tensor("bo_p", [H, 1], F32,
                               kind="ExternalInput").ap(),
        "bo_h": nc.dram_tensor("bo_h", [H, 1], F32,
                               kind="ExternalInput").ap(),
        "xqf": nc.dram_tensor("xqf", [NCHUNK, NBLK, 6, BLK * 3 * BC], FP16,
                              kind="ExternalInput").ap(),
        "xqo": nc.dram_tensor("xqo", [NCHUNK, NBLK, 1, BLK * BC], FP16,
                              kind="ExternalInput").ap(),
        "preds": nc.dram_tensor("preds", [NEP, NCHUNK, 32, 4 * BC], F32,
                                kind="ExternalOutput").ap(),
    }
    with tile.TileContext(nc) as tc:
        _build_body(tc, d, NP, NH)
    nc.compile()
    return nc


def _host_prep(y_flow, W_ih, W_hh, b_ih, b_hh, W_out, b_out, NP):
    """Build per-core input maps. y_flow: (B, T, 1) f32."""
    f16 = np.float16
    W_ih = np.asarray(W_ih, np.float32)
    W_hh = np.asarray(W_hh, np.float32)
    W_out = np.asarray(W_out, np.float32)
    bias = np.asarray(b_ih, np.float32) + np.asarray(b_hh, np.float32)
    b_out = np.asarray(b_out, np.float32)

    W_eff = W_hh + W_ih @ W_out           # [4H, H] (phase-H feedback fold)
    b_eff = bias + W_ih[:, 0] * b_out[0]

    sc = _SCALE[:, None]
    whhT_p = np.ascontiguousarray((W_hh[_PERM] * sc).T).astype(f16)
    whhT_h = np.ascontiguousarray((W_eff[_PERM] * sc).T).astype(f16)

    wih_s = (W_ih[_PERM, 0] * _SCALE_B).astype(np.float32)
    b_s = (bias[_PERM] * _SCALE_B).astype(np.float32)
    beff_s = (b_eff[_PERM] * _SCALE_B).astype(np.float32)

    bp6 = np.zeros((6, H), np.float32)
    bpo = wih_s[3 * H:4 * H].reshape(1, H).astype(np.float32)
    bh4 = np.zeros((4, H), np.float32)
    ones4 = np.zeros((4, 4 * BC), np.float32)
    for j in range(4):
        if j < 3:
            bp6[2 * j] = wih_s[j * H:(j + 1) * H]
            bp6[2 * j + 1] = b_s[j * H:(j + 1) * H]
        bh4[j] = beff_s[j * H:(j + 1) * H]
        ones4[j, j * BC:(j + 1) * BC] = 1.0

    woutZ = np.zeros((H, 2 * H), np.float32)
    woutZ[:, H] = W_out[0]
    bo_p = b_s[3 * H:4 * H].reshape(H, 1).astype(np.float32)
    bo_h = beff_s[3 * H:4 * H].reshape(H, 1).astype(np.float32)

    NBLK = (NP + BLK - 1) // BLK
    NPAD = NBLK * BLK
    y = np.asarray(y_flow, np.float32)[:, :, 0]                   # [B, T]
    in_maps = []
    for core in range(NCORES):
        yc = y[core * BS:(core + 1) * BS]                         # [BS, T]
        xqf = np.zeros((NCHUNK, NPAD, 6, 3 * BC), np.float32)
        xqo = np.zeros((NCHUNK, NPAD, 1, BC), np.float32)
        for ch in range(NCHUNK):
            ystep = yc[ch * BC:(ch + 1) * BC, :NP].T              # [NP, BC]
            for j in range(3):
                xqf[ch, :NP, 2 * j, j * BC:(j + 1) * BC] = ystep
                xqf[ch, :, 2 * j + 1, j * BC:(j + 1) * BC] = 1.0
            xqo[ch, :NP, 0, :] = ystep
        xqf = xqf.reshape(NCHUNK, NBLK, BLK, 6, 3 * BC)
        xqf = np.ascontiguousarray(xqf.transpose(0, 1, 3, 2, 4))
        xqf = xqf.reshape(NCHUNK, NBLK, 6, BLK * 3 * BC)
        xqo = xqo.reshape(NCHUNK, NBLK, BLK, 1, BC)
        xqo = np.ascontiguousarray(xqo.transpose(0, 1, 3, 2, 4))
        xqo = xqo.reshape(NCHUNK, NBLK, 1, BLK * BC)
        in_maps.append({
            "whhT_p": whhT_p, "whhT_h": whhT_h,
            "bp6": bp6.astype(f16), "bpo": bpo.astype(f16),
            "bh4": bh4.astype(f16),
            "ones4": ones4.astype(f16), "woutZ": woutZ.astype(f16),
            "bo_p": bo_p, "bo_h": bo_h,
            "xqf": xqf.astype(f16), "xqo": xqo.astype(f16),
        })
    return in_maps


def kernel(y_flow, x_dyn, W_ih, W_hh, b_ih, b_hh, W_out, b_out, twin_idx,
           _trace=False):
    twin = int(twin_idx)
    assert twin == 256, f"kernel hardcodes twin_idx=256, got {twin}"
    B, T, _ = y_flow.shape
    assert (B, T) == (2048, 512)
    NP, NH = twin - 1, T - twin
    NT = NP + NH

    nc = _program(NP, NH)
    in_maps = _host_prep(y_flow, W_ih, W_hh, b_ih, b_hh, W_out, b_out, NP)
    res = run_bass_kernel_spmd(nc, in_maps, core_ids=list(range(NCORES)),
                               trace=_trace)

    b_out = np.asarray(b_out, np.float32)
    out = np.empty((B, NT, 1), np.float32)
    for core in range(NCORES):
        p = np.asarray(res.results[core]["preds"], np.float32)
        nep = p.shape[0]
        a = p.reshape(nep, NCHUNK, 32, 4, BC)      # [e, ch, r, j, b]
        for ch in range(NCHUNK):
            blk = a[:, ch].transpose(3, 0, 1, 2).reshape(BC, -1)[:, :NT]
            out[core * BS + ch * BC: core * BS + (ch + 1) * BC, :, 0] = \
                blk + b_out[0]
    if _trace:
        kernel._last_results = res
    return out


# revision 44
# speedup vs baseline: 1.2084x; 1.0077x over previous
"""Bass/Trainium2 kernel for nn_BaselineLSTM (B=2048, T=512, H=128, twin=256).

Strategy (final):
  - Data-parallel: batch 2048 -> 8 cores x 256; each core runs 2 interleaved
    chunks of 128 batch. The kernel is bound three ways at ~2.9us/step
    (per-chunk serial chain ~ PE busy ~ ACT busy), so every stage below is
    tuned to keep all three at their floor.
  - State kept transposed: h/c = [H=128 partitions, batch free]; cT = 2c so
    the g-gate tanh becomes a sigmoid: tanh(x) = 2*sigmoid(2x)-1, with the
    2x folded into the g-rows of the weights. tanh(c) itself is computed as
    Tanh(scale=0.5) on cT, so h = tanh(c)*sigma(o) is a plain multiply.
  - Gate order [i|f|g|o]: ONE sigmoid covers [i,f,g] (on the serial chain);
    the o-gate lives in its OWN psum bank (double-buffered; psum bank-level
    serialization would otherwise make sigma_ifg wait the o matmul too) with
    its own sigmoid OFF the chain, its bias carried by the activation's
    per-partition bias AP, and in phase H no input matmul at all.
  - i/f/g input + bias enter via ONE K=6 (phase P) / K=3 (phase H) matmul
    with a block-diagonal rhs (phase P rhs streamed from DRAM, phase H rhs
    static), accumulated into the gates bank before the 3 recurrent matmuls;
    phase P feeds the o-gate with one K=1 matmul.
  - Cell update on DVE only (gpsimd is pathologically slow for elementwise):
      t2 = sf*cT;  u = (s2g-0.5)*si;  cT = 4u + t2;  h = tanh(cT/2)*so.
  - fp16 everywhere on-chip (not bf16): the 2*sigmoid(2x)-1 rewrite loses
    absolute precision near 0.5 in bf16; fp16's 10 mantissa bits restore it,
    and fp16 keeps the DVE 2x packed modes.
  - Predictions p_t = W_out h_t (+ b_out on host): h kept in an 8-slot ring;
    one shifted-stationary matmul per 4 steps per chunk accumulates 128
    steps into one PSUM bank, flushed to DRAM per 128-step epoch. The 8-slot
    ring lets the pred matmul run 1-2 steps late, emitted into a PE-queue
    slot measured to be stall-free (the in-order PE queue head-blocks on
    any instruction whose inputs are not ready yet).
  - The PE runs at 1.2 GHz on this part no matter what (sustained dense
    matmul bursts never unthrottle it), so matmul costs are budgeted cold.
"""

import functools

import numpy as np

import concourse.bacc as bacc
import concourse.tile as tile
from concourse import mybir
from concourse.bass_utils import run_bass_kernel_spmd

F32 = mybir.dt.float32
FP16 = mybir.dt.float16
AF = mybir.ActivationFunctionType
OP = mybir.AluOpType

H = 128          # hidden
NCORES = 8
BS = 256         # batch per core
BC = 128         # batch per chunk
NCHUNK = 2
BLK = 32         # xq steps per DMA block

# kernel gate order == pytorch order (i, f, g, o): sigma(i,f,g) is one
# contiguous on-chain activation; sigma(o) is separate and off-chain (o is
# first needed only after tanh(c)).
_PERM = np.arange(512)
# g-gate rows doubled so the matmul emits 2*pre_g for the
# tanh(x) = 2*sigmoid(2x)-1 rewrite; cT state = 2c.
_SCALE = np.repeat([1.0, 1.0, 2.0, 1.0], 128)
_SCALE_B = _SCALE


def _build_body(tc, d, NP, NH):
    nc = tc.nc
    NT = NP + NH
    NBLK = (NP + BLK - 1) // BLK

    import contextlib
    with contextlib.ExitStack() as ctx:
        consts = ctx.enter_context(tc.tile_pool(name="consts", bufs=1))
        state = ctx.enter_context(tc.tile_pool(name="state", bufs=1))
        spool = ctx.enter_context(tc.tile_pool(name="sig", bufs=4))
        wpool = ctx.enter_context(tc.tile_pool(name="work", bufs=4))
        xpool = ctx.enter_context(tc.tile_pool(name="xq", bufs=2))
        gpool = ctx.enter_context(tc.tile_pool(name="gates", bufs=1, space="PSUM"))
        opool = ctx.enter_context(tc.tile_pool(name="ogate", bufs=2, space="PSUM"))
        ppool = ctx.enter_context(tc.tile_pool(name="ppsum", bufs=1, space="PSUM"))

        # ---- constants to SBUF
        whhT_p = consts.tile([H, 4 * H], FP16, tag="whhT_p")
        whhT_h = consts.tile([H, 4 * H], FP16, tag="whhT_h")
        bp6 = consts.tile([6, H], FP16, tag="bp6")
        bpo = consts.tile([1, H], FP16, tag="bpo")
        bh4 = consts.tile([4, H], FP16, tag="bh4")
        ones4 = consts.tile([4, 4 * BC], FP16, tag="ones4")
        woutZ = consts.tile([H, 2 * H], FP16, tag="woutZ")
        bo_p = consts.tile([H, 1], F32, tag="bo_p")
        bo_h = consts.tile([H, 1], F32, tag="bo_h")
        nc.sync.dma_start(out=whhT_p, in_=d["whhT_p"])
        nc.sync.dma_start(out=whhT_h, in_=d["whhT_h"])
        nc.sync.dma_start(out=bp6, in_=d["bp6"])
        nc.sync.dma_start(out=bpo, in_=d["bpo"])
        nc.sync.dma_start(out=bh4, in_=d["bh4"])
        nc.sync.dma_start(out=ones4, in_=d["ones4"])
        nc.sync.dma_start(out=woutZ, in_=d["woutZ"])
        nc.sync.dma_start(out=bo_p, in_=d["bo_p"])
        nc.sync.dma_start(out=bo_h, in_=d["bo_h"])

        # ---- state
        hist = []
        cT = []
        for ch in range(NCHUNK):
            hh = state.tile([H, 8 * BC], FP16, tag=f"hist{ch}")
            c = state.tile([H, BC], FP16, tag=f"cT{ch}")
            nc.vector.memset(hh, 0.0)
            nc.vector.memset(c, 0.0)
            hist.append(hh)
            cT.append(c)

        # ---- xq streams (phase P block-diag rhs), double buffered
        xftiles = [[None] * NBLK for _ in range(NCHUNK)]
        xotiles = [[None] * NBLK for _ in range(NCHUNK)]

        def fetch(blk):
            for ch in range(NCHUNK):
                xf = xpool.tile([6, BLK * 3 * BC], FP16, tag=f"xqf{ch}",
                                name=f"xqf{ch}_{blk}")
                nc.sync.dma_start(out=xf, in_=d["xqf"][ch, blk])
                xftiles[ch][blk] = xf
                xo = xpool.tile([1, BLK * BC], FP16, tag=f"xqo{ch}",
                                name=f"xqo{ch}_{blk}")
                nc.sync.dma_start(out=xo, in_=d["xqo"][ch, blk])
                xotiles[ch][blk] = xo

        fetch(0)
        fetch(1)

        s4s = [None, None]
        sos = [None, None]
        ogs = [None]
        pps = [None, None]

        def front(s, ch):
            """gate MMs + sigmoids.  The o-gate lives in its OWN psum bank:
            sigma(i,f,g) then only waits 3 recurrent MMs (psum bank-level
            serialization would otherwise make it wait the o MM too), the
            o-gate bias rides on sigma_o's per-partition bias AP, and the
            block-diagonal bias/input stream shrinks to 384 columns."""
            phase_p = s < NP
            gates = gpool.tile([H, 3 * BC], F32, tag=f"g{ch}",
                               name=f"g{ch}_{s}")
            if phase_p:
                og = opool.tile([H, BC], F32, tag=f"o{ch}",
                                name=f"o{ch}_{s}")
            elif ch == 0:
                ogs[0] = opool.tile([H, 2, BC], F32, tag="o0",
                                    name=f"osh_{s}")
            if phase_p:
                blk, sl = divmod(s, BLK)
                nc.tensor.matmul(gates, bp6,
                                 xftiles[ch][blk][:, sl * 3 * BC:
                                                  (sl + 1) * 3 * BC],
                                 start=True, stop=False,
                                 skip_group_check=True)
                nc.tensor.matmul(og, bpo,
                                 xotiles[ch][blk][:, sl * BC:(sl + 1) * BC],
                                 start=True, stop=False,
                                 skip_group_check=True)
            else:
                nc.tensor.matmul(gates, bh4[0:3, :], ones4[0:3, 0:3 * BC],
                                 start=True, stop=False,
                                 skip_group_check=True)
            whh = whhT_p if phase_p else whhT_h
            hprev = hist[ch][:, ((s - 1) % 8) * BC: ((s - 1) % 8 + 1) * BC]
            for j in range(3):
                nc.tensor.matmul(gates[:, j * H:(j + 1) * H],
                                 whh[:, j * H:(j + 1) * H], hprev,
                                 start=False, stop=(j == 2),
                                 skip_group_check=True)
            if phase_p:
                nc.tensor.matmul(og, whh[:, 3 * H:4 * H], hprev,
                                 start=False, stop=True,
                                 skip_group_check=True)
            else:
                # phase H: both chunks' o-gates share ONE bank (each chunk
                # is a single start=True matmul into its half, so write
                # order between chunks cannot corrupt the bank) so that ONE
                # sigma covers both -- saves one ACT instruction per step
                # on the near-saturated Scalar engine.
                nc.tensor.matmul(ogs[0][:, ch, :], whh[:, 3 * H:4 * H],
                                 hprev, start=True, stop=True,
                                 skip_group_check=True)
            s4 = spool.tile([H, 3 * BC], FP16, tag=f"s4{ch}",
                            name=f"s4{ch}_{s}")
            nc.scalar.activation(s4, gates, AF.Sigmoid)
            if phase_p:
                so = spool.tile([H, BC], FP16, tag=f"so{ch}",
                                name=f"so{ch}_{s}")
                nc.scalar.activation(so, og, AF.Sigmoid, bias=bo_p[:, 0:1])
                sos[ch] = so
            elif ch == 1:
                so2 = spool.tile([H, 2, BC], FP16, tag="sosh",
                                 name=f"sosh_{s}")
                nc.scalar.activation(so2, ogs[0], AF.Sigmoid,
                                     bias=bo_h[:, 0:1])
                sos[0] = so2[:, 0, :]
                sos[1] = so2[:, 1, :]
            s4s[ch] = s4

        def back(s, ch):
            """cell update on DVE + sigma(cT) + hT + batched prediction MM."""
            s4 = s4s[ch]
            t2 = wpool.tile([H, BC], FP16, tag=f"t2{ch}", name=f"t2{ch}_{s}")
            nc.vector.tensor_mul(t2, s4[:, H:2 * H], cT[ch])
            u = wpool.tile([H, BC], FP16, tag=f"u{ch}", name=f"u{ch}_{s}")
            nc.vector.scalar_tensor_tensor(u, s4[:, 2 * H:3 * H], 0.5,
                                           s4[:, 0:H], OP.subtract, OP.mult)
            nc.vector.scalar_tensor_tensor(cT[ch], u, 4.0, t2,
                                           OP.mult, OP.add)
            tc_ = wpool.tile([H, BC], FP16, tag=f"sc{ch}", name=f"sc{ch}_{s}")
            nc.scalar.activation(tc_, cT[ch], AF.Tanh, scale=0.5)
            hslot = hist[ch][:, (s % 8) * BC: (s % 8 + 1) * BC]
            nc.vector.tensor_mul(hslot, tc_, sos[ch])


        def emit_pred(G, ch):
            NG = (NT + 3) // 4
            r = G % 32
            n = min(NT - 4 * G, 4) * BC
            base = (G % 2) * 4 * BC
            if r == 0:
                pps[ch] = ppool.tile([H, 4 * BC], F32, tag=f"pps{ch}",
                                     name=f"pps{ch}_{G}")
            nc.tensor.matmul(pps[ch][:, 0:n],
                             woutZ[:, H - r: 2 * H - r],
                             hist[ch][:, base: base + n],
                             start=(r == 0), stop=(r == 31 or G == NG - 1),
                             skip_group_check=True)
            if r == 31 or G == NG - 1:
                e = G // 32
                pc = wpool.tile([32, 4 * BC], F32, tag=f"pc{ch}",
                                name=f"pc{ch}_{G}")
                nc.vector.tensor_copy(pc, pps[ch][0:32, :])
                nc.sync.dma_start(out=d["preds"][e, ch], in_=pc)

        # Software pipeline: full A-step then full B-step per iteration.
        # Each engine's FIFO then alternates A-stage / B-stage, which locks
        # the two chunks half a step out of phase (emitting both fronts
        # together lets the chunks drift in-phase and exposes the full
        # serial chain latency).
        for s in range(NT):
            if s % BLK == BLK // 2:
                nb = s // BLK + 2
                if nb < NBLK:
                    fetch(nb)
            if s < NP:
                front(s, 0)
                back(s, 0)
            else:
                front(s, 0)
                front(s, 1)
                back(s, 0)
            # Predictions (group G = steps 4G..4G+3) are emitted 1-2 steps
            # after the group completes, always in the PE-queue slot right
            # after chunk 0's gate MMs (measured stall-free: the pred matmul
            # is ready by then and fills the mid-step PE window).
            if s % 4 == 0 and s >= 4:
                emit_pred(s // 4 - 1, 0)
            if s % 4 == 2 and s >= 6:
                emit_pred((s - 2) // 4 - 1, 1)
            if s < NP:
                front(s, 1)
            back(s, 1)

        # final prediction group(s) not yet emitted by the loop
        NG = (NT + 3) // 4
        emit_pred(NG - 1, 0)
        emit_pred(NG - 1, 1)


@functools.lru_cache(maxsize=2)
def _program(NP, NH):
    nc = bacc.Bacc("TRN2", target_bir_lowering=False, debug=False,
                   num_devices=NCORES)
    NT = NP + NH
    NEP = (NT + 127) // 128
    NBLK = (NP + BLK - 1) // BLK
    d = {
        "whhT_p": nc.dram_tensor("whhT_p", [H, 4 * H], FP16,
                                 kind="ExternalInput").ap(),
        "whhT_h": nc.dram_tensor("whhT_h", [H, 4 * H], FP16,
                                 kind="ExternalInput").ap(),
        "bp6": nc.dram_tensor("bp6", [6, H], FP16, kind="ExternalInput").ap(),
        "bpo": nc.dram_tensor("bpo", [1, H], FP16, kind="ExternalInput").ap(),
        "bh4": nc.dram_tensor("bh4", [4, H], FP16, kind="ExternalInput").ap(),
        "ones4": nc.dram_tensor("ones4", [4, 4 * BC], FP16,
                                kind="ExternalInput").ap(),
        "woutZ": nc.dram_tensor("woutZ", [H, 2 * H], FP16,
                                kind="ExternalInput").ap(),
        "bo_p": nc.dram_tensor("bo_p", [H, 1], F32,
                               kind="ExternalInput").ap(),
        "bo_h": nc.dram_tensor("bo_h", [H, 1], F32,
                               kind="ExternalInput").ap(),
        "xqf": nc.dram_tensor("xqf", [NCHUNK, NBLK, 6, BLK * 3 * BC], FP16,
                              kind="ExternalInput").ap(),
        "xqo": nc.dram_tensor("xqo", [NCHUNK, NBLK, 1, BLK * BC], FP16,
                              kind="ExternalInput").ap(),
        "preds": nc.dram_tensor("preds", [NEP, NCHUNK, 32, 4 * BC], F32,
                                kind="ExternalOutput").ap(),
    }
    with tile.TileContext(nc) as tc:
        _build_body(tc, d, NP, NH)
    nc.compile()
    return nc


def _host_prep(y_flow, W_ih, W_hh, b_ih, b_hh, W_out, b_out, NP):
    """Build per-core input maps. y_flow: (B, T, 1) f32."""
    f16 = np.float16
    W_ih = np.asarray(W_ih, np.float32)
    W_hh = np.asarray(W_hh, np.float32)
    W_out = np.asarray(W_out, np.float32)
    bias = np.asarray(b_ih, np.float32) + np.asarray(b_hh, np.float32)
    b_out = np.asarray(b_out, np.float32)

    W_eff = W_hh + W_ih @ W_out           # [4H, H] (phase-H feedback fold)
    b_eff = bias + W_ih[:, 0] * b_out[0]

    sc = _SCALE[:, None]
    whhT_p = np.ascontiguousarray((W_hh[_PERM] * sc).T).astype(f16)
    whhT_h = np.ascontiguousarray((W_eff[_PERM] * sc).T).astype(f16)

    wih_s = (W_ih[_PERM, 0] * _SCALE_B).astype(np.float32)
    b_s = (bias[_PERM] * _SCALE_B).astype(np.float32)
    beff_s = (b_eff[_PERM] * _SCALE_B).astype(np.float32)

    bp6 = np.zeros((6, H), np.float32)
    bpo = wih_s[3 * H:4 * H].reshape(1, H).astype(np.float32)
    bh4 = np.zeros((4, H), np.float32)
    ones4 = np.zeros((4, 4 * BC), np.float32)
    for j in range(4):
        if j < 3:
            bp6[2 * j] = wih_s[j * H:(j + 1) * H]
            bp6[2 * j + 1] = b_s[j * H:(j + 1) * H]
        bh4[j] = beff_s[j * H:(j + 1) * H]
        ones4[j, j * BC:(j + 1) * BC] = 1.0

    woutZ = np.zeros((H, 2 * H), np.float32)
    woutZ[:, H] = W_out[0]
    bo_p = b_s[3 * H:4 * H].reshape(H, 1).astype(np.float32)
    bo_h = beff_s[3 * H:4 * H].reshape(H, 1).astype(np.float32)

    NBLK = (NP + BLK - 1) // BLK
    NPAD = NBLK * BLK
    y = np.asarray(y_flow, np.float32)[:, :, 0]                   # [B, T]
    in_maps = []
    for core in range(NCORES):
        yc = y[core * BS:(core + 1) * BS]                         # [BS, T]
        xqf = np.zeros((NCHUNK, NPAD, 6, 3 * BC), np.float32)
        xqo = np.zeros((NCHUNK, NPAD, 1, BC), np.float32)
        for ch in range(NCHUNK):
            ystep = yc[ch * BC:(ch + 1) * BC, :NP].T              # [NP, BC]
            for j in range(3):
                xqf[ch, :NP, 2 * j, j * BC:(j + 1) * BC] = ystep
                xqf[ch, :, 2 * j + 1, j * BC:(j + 1) * BC] = 1.0
            xqo[ch, :NP, 0, :] = ystep
        xqf = xqf.reshape(NCHUNK, NBLK, BLK, 6, 3 * BC)
        xqf = np.ascontiguousarray(xqf.transpose(0, 1, 3, 2, 4))
        xqf = xqf.reshape(NCHUNK, NBLK, 6, BLK * 3 * BC)
        xqo = xqo.reshape(NCHUNK, NBLK, BLK, 1, BC)
        xqo = np.ascontiguousarray(xqo.transpose(0, 1, 3, 2, 4))
        xqo = xqo.reshape(NCHUNK, NBLK, 1, BLK * BC)
        in_maps.append({
            "whhT_p": whhT_p, "whhT_h": whhT_h,
            "bp6": bp6.astype(f16), "bpo": bpo.astype(f16),
            "bh4": bh4.astype(f16),
            "ones4": ones4.astype(f16), "woutZ": woutZ.astype(f16),
            "bo_p": bo_p, "bo_h": bo_h,
            "xqf": xqf.astype(f16), "xqo": xqo.astype(f16),
        })
    return in_maps


def kernel(y_flow, x_dyn, W_ih, W_hh, b_ih, b_hh, W_out, b_out, twin_idx,
           _trace=False):
    twin = int(twin_idx)
    assert twin == 256, f"kernel hardcodes twin_idx=256, got {twin}"
    B, T, _ = y_flow.shape
    assert (B, T) == (2048, 512)
    NP, NH = twin - 1, T - twin
    NT = NP + NH

    nc = _program(NP, NH)
    in_maps = _host_prep(y_flow, W_ih, W_hh, b_ih, b_hh, W_out, b_out, NP)
    res = run_bass_kernel_spmd(nc, in_maps, core_ids=list(range(NCORES)),
                               trace=_trace)

    b_out = np.asarray(b_out, np.float32)
    out = np.empty((B, NT, 1), np.float32)
    for core in range(NCORES):
        p = np.asarray(res.results[core]["preds"], np.float32)
        nep = p.shape[0]
        a = p.reshape(nep, NCHUNK, 32, 4, BC)      # [e, ch, r, j, b]
        for ch in range(NCHUNK):
            blk = a[:, ch].transpose(3, 0, 1, 2).reshape(BC, -1)[:, :NT]
            out[core * BS + ch * BC: core * BS + (ch + 1) * BC, :, 0] = \
                blk + b_out[0]
    if _trace:
        kernel._last_results = res
    return out
